# revision 29
# baseline (speedup 1.0000x reference)
"""Trainium2 Bass kernel for LocalGlobalSelfAttention (v3).

Sharding: 8 cores = 4 batches x 2 sequence-halves (no collectives).

v3 changes vs v2:
  - All six input projections and the output projection run in fp8e4m3
    with perf_mode=DoubleRow (contraction pairs fused, half the matmul
    instructions). Weights are host-scaled x32 into fp8's normal range;
    the descale folds into existing ops for free: exp scale is exactly
    2^-13 (=0.125/1024 for the 32x32 q/k scaling), bq/bv host-scaled
    x32, the softmax reciprocal is scaled x2 (so oh = 64*o_norm sits in
    fp8 range), wo host-scaled x32, and the residual xq is host-scaled
    x2048 with LN eps scaled x2048^2 (LayerNorm is scale-invariant).
  - Flat global-lag attention loop: AV/evac of head-pair h interleaves
    with the first score rounds of head-pair h+1 (no per-hp ScalarE
    hiccup).
v2 recap: deferred softmax normalization (batched reciprocal), o kept
in SBUF, shared AV PSUM tile with col-tiled concurrent subheads +
ones-matmul rowsums, bk/bo dropped (mathematically cancel), projection
chunk chains with warmup attention rounds interleaved.
"""

import numpy as np
import ml_dtypes
from contextlib import ExitStack

BF16 = ml_dtypes.bfloat16
FP8 = ml_dtypes.float8_e4m3

FULL_CFG = dict(S=2048, D=1024, H=16, K=64, NW=8)
N_CORES = 8
LN_EPS = 1e-3
WSCALE = 32.0
YSCALE = 2048.0  # WSCALE(oh=2*32) * WSCALE(wo)


def _chunks(total, size):
    return [(o, min(size, total - o)) for o in range(0, total, size)]


def build_nc(cfg=None):
    import concourse.bass as bass
    import concourse.tile as tile
    import concourse.mybir as mybir
    from concourse import bacc

    cfg = dict(cfg or FULL_CFG)
    S, D, H, K, NW = cfg["S"], cfg["D"], cfg["H"], cfg["K"], cfg["NW"]
    HK = H * K
    SH = S // 2
    WIN = S // NW
    NWH = SH // WIN
    assert K == 64 and D % 256 == 0 and HK % 256 == 0

    ND = D // 128
    NP = D // 256        # contraction pair-tiles
    NHK = HK // 128
    NST = S // 128
    NQT = SH // 128
    NSS = WIN // 128

    f32 = mybir.dt.float32
    bf16 = mybir.dt.bfloat16
    fp8 = mybir.dt.float8e4
    DR = mybir.MatmulPerfMode.DoubleRow
    Exp = mybir.ActivationFunctionType.Exp
    Square = mybir.ActivationFunctionType.Square
    Sqrt = mybir.ActivationFunctionType.Sqrt
    add_op = mybir.AluOpType.add
    mult_op = mybir.AluOpType.mult
    sub_op = mybir.AluOpType.subtract
    AxX = mybir.AxisListType.X

    nc = bacc.Bacc("TRN2", target_bir_lowering=False, debug=False,
                   num_devices=N_CORES)

    # ---- DRAM parameters -------------------------------------------------
    xti_d = nc.dram_tensor("xti", [NP, 128, 2, S], fp8, kind="ExternalInput")
    xq_d = nc.dram_tensor("xq", [SH, D], f32, kind="ExternalInput")
    w_d = {}
    for nm in ("wq_g", "wk_g", "wv_g", "wq_l", "wk_l", "wv_l"):
        w_d[nm] = nc.dram_tensor(nm, [NP, 128, 2, HK], fp8,
                                 kind="ExternalInput")
    wo_d = nc.dram_tensor("wo", [NHK, 128, 2, D], fp8, kind="ExternalInput")
    bcol_d = {}
    for nm in ("bq_g", "bq_l"):
        bcol_d[nm] = nc.dram_tensor(nm, [NHK, 128], f32, kind="ExternalInput")
    bv_g_d = nc.dram_tensor("bv_g", [1, HK], bf16, kind="ExternalInput")
    bv_l_d = nc.dram_tensor("bv_l", [1, HK], bf16, kind="ExternalInput")
    gamma_d = nc.dram_tensor("gamma", [1, D], f32, kind="ExternalInput")
    beta_d = nc.dram_tensor("beta", [1, D], f32, kind="ExternalInput")
    out_d = nc.dram_tensor("out", [SH, D], f32, kind="ExternalOutput")
    # slot rows: global (hp,sub) at 2hp+sub (0..15), local at 32+2hp+sub
    rs_d = nc.dram_tensor("rs_scr", [6 * NHK, SH], f32)
    ri_d = nc.dram_tensor("ri_scr", [6 * NHK, SH], bf16)

    PS = bass.MemorySpace.PSUM
    WARMUP = 2
    LAG = 2

    g_rounds = [([(0, SH, stt * 128, stt)], None) for stt in range(NST)]
    l_rounds = [([(w * WIN, WIN, (w * NSS + ss) * 128, w * NSS + ss)
                  for w in range(NWH)], None) for ss in range(NSS)]

    def bank_plan(rounds):
        first_b, last_b = {}, {}
        for rd, (segs, _) in enumerate(rounds):
            for (qo, ql, _sc, _vx) in segs:
                for co, cl in _chunks(ql, 512):
                    bank = (qo + co) // 512
                    first_b.setdefault(bank, (rd, qo + co))
                    last_b[bank] = (rd, qo + co)
        return first_b, last_b

    g_first, g_last = bank_plan(g_rounds)
    l_first, l_last = bank_plan(l_rounds)

    with tile.TileContext(nc) as tc, ExitStack() as ctx:
        cpool = ctx.enter_context(tc.tile_pool(name="consts", bufs=1))
        rsp = ctx.enter_context(tc.tile_pool(name="rs", bufs=2))
        ogp = ctx.enter_context(tc.tile_pool(name="og", bufs=1))
        olp = ctx.enter_context(tc.tile_pool(name="ol", bufs=1))
        rip = ctx.enter_context(tc.tile_pool(name="ri", bufs=1))
        wop = ctx.enter_context(tc.tile_pool(name="wo2", bufs=1))
        expp = scp0 = scp1 = None

        ones_col = cpool.tile([128, 1], bf16, tag="onesc", name="onesc")
        nc.vector.memset(ones_col[:], 1.0)
        eps_col = cpool.tile([128, 1], f32, tag="eps", name="eps")
        nc.vector.memset(eps_col[:], float(LN_EPS * YSCALE * YSCALE))
        bcol_sb = {}
        for nm, dten in bcol_d.items():
            cols = []
            for j in range(NHK):
                t = cpool.tile([128, 1], f32, tag=f"{nm}{j}", name=f"{nm}{j}")
                nc.sync.dma_start(t[:], dten[j, :].rearrange("(a b) -> a b", b=1))
                cols.append(t)
            bcol_sb[nm] = cols
        bv_bc = {}
        for sname, dten in (("g", bv_g_d), ("l", bv_l_d)):
            t = cpool.tile([128, HK], bf16, tag=f"bvbc{sname}", name=f"bvbc{sname}")
            nc.sync.dma_start(t[:], dten[:].partition_broadcast(128))
            bv_bc[sname] = t

        # ---- attention emission ------------------------------------------
        state = dict(ex={}, o_ps=None, rs_ps=None)

        def emit_scores(kT, qT, hp, rounds, rd):
            segs, _ = rounds[rd]
            sc = [scp0.tile([128, SH], f32, tag="sc0", name="sc0"),
                  scp1.tile([128, SH], f32, tag="sc1", name="sc1")]
            for (qo, ql, scol, _v) in segs:
                for co, cl in _chunks(ql, 512):
                    for sub in range(2):
                        po = sub * 64
                        nc.tensor.matmul(
                            sc[sub][:, qo + co:qo + co + cl],
                            kT[hp][po:po + 64, scol:scol + 128],
                            qT[hp][po:po + 64, qo + co:qo + co + cl],
                            start=True, stop=True)
            ex = []
            for sub in range(2):
                e = expp.tile([128, SH], bf16, tag=f"ex{sub}", name=f"ex{sub}")
                # q,k carry x32 each -> scores x1024; softmax scale 1/8
                nc.scalar.activation(e[:], sc[sub][:], Exp, scale=2.0 ** -13)
                ex.append(e)
            state["ex"][(hp, rd)] = ex

        def emit_av(vx, hp, rounds, rd, first_b, last_b):
            segs, _ = rounds[rd]
            ex = state["ex"].pop((hp, rd))
            o_ps, rs_ps = state["o_ps"], state["rs_ps"]
            for (qo, ql, _scol, vxt) in segs:
                for co, cl in _chunks(ql, 512):
                    col = qo + co
                    bank = col // 512
                    start = first_b[bank] == (rd, col)
                    stop = last_b[bank] == (rd, col)
                    for sub in range(2):
                        h = 2 * hp + sub
                        nc.tensor.matmul(
                            o_ps[sub * 64:sub * 64 + 64, col:col + cl],
                            vx[vxt][:, h, :], ex[sub][:, col:col + cl],
                            start=start, stop=stop,
                            tile_position=(0, sub * 64))
                    for sub in range(2):
                        nc.tensor.matmul(
                            rs_ps[sub * 32:sub * 32 + 1, col:col + cl],
                            ones_col[:, 0:1], ex[sub][:, col:col + cl],
                            start=start, stop=stop,
                            tile_position=(0, sub * 32))

        def run_attention(kT, qT, vx, rounds, first_b, last_b, o_sb,
                          rs_base, opp, rpp, skip_scores=0, lag=LAG):
            nr = len(rounds)
            seq = [(hp, rd) for hp in range(NHK) for rd in range(nr)]
            n = len(seq)
            for i in range(n + lag):
                if i >= lag:
                    hp, rd = seq[i - lag]
                    if rd == 0:
                        state["o_ps"] = opp.tile([128, SH], f32, tag="ops",
                                                 name="ops")
                        state["rs_ps"] = rpp.tile([33, SH], f32, tag="rps",
                                                  name="rps")
                    emit_av(vx, hp, rounds, rd, first_b, last_b)
                    if rd == nr - 1:
                        stg = rsp.tile([33, SH], f32, tag="rstg", name="rstg")
                        nc.vector.tensor_copy(stg[:], state["rs_ps"][:])
                        for sub in range(2):
                            r = rs_base + 2 * hp + sub
                            nc.sync.dma_start(
                                rs_d[r, :].rearrange("(a f) -> a f", a=1),
                                stg[sub * 32:sub * 32 + 1, :])
                        nc.vector.tensor_copy(o_sb[hp][:], state["o_ps"][:])
                if i < n:
                    hp, rd = seq[i]
                    if not (hp == 0 and rd < skip_scores):
                        emit_scores(kT, qT, hp, rounds, rd)

        # ================= Phase A + B + A2 ==============================
        with tc.tile_pool(name="xin", bufs=1) as xin, \
             tc.tile_pool(name="wt", bufs=2) as wt, \
             tc.tile_pool(name="exp", bufs=4) as _expp, \
             tc.tile_pool(name="sc0", bufs=1, space=PS) as _scp0, \
             tc.tile_pool(name="sc1", bufs=1, space=PS) as _scp1:
            expp, scp0, scp1 = _expp, _scp0, _scp1

            xti_sb = [xin.tile([128, 2, S], fp8, tag=f"xt{pp}", name=f"xt{pp}")
                      for pp in range(NP)]
            for pp in range(NP):
                nc.sync.dma_start(xti_sb[pp][:], xti_d[pp])

            def load_w(nm):
                ts = []
                for pp in range(NP):
                    t = wt.tile([128, 2, HK], fp8, tag=f"wp{pp}", name=f"wp{pp}")
                    nc.sync.dma_start(t[:], w_d[nm][pp])
                    ts.append(t)
                return ts

            def kq_chain(pool, w_tiles, j, so, sl, out_tile, bias):
                pt = pool.tile([128, 512], f32, tag="pt", name="pt")
                for pp in range(NP):
                    nc.tensor.matmul(pt[:, 0:sl],
                                     w_tiles[pp][:, :, j * 128:(j + 1) * 128],
                                     xti_sb[pp][:, :, so:so + sl],
                                     start=(pp == 0), stop=(pp == NP - 1),
                                     perf_mode=DR)
                if bias is None:
                    nc.vector.tensor_copy(out_tile[:, so:so + sl], pt[:, 0:sl])
                else:
                    nc.vector.tensor_scalar(out_tile[:, so:so + sl],
                                            pt[:, 0:sl], bias, None, add_op)

            def v_chain(pool, w_tiles, t, ho, hl, out_tiles, bvt):
                pt = pool.tile([128, 512], f32, tag="pt", name="pt")
                for pp in range(NP):
                    nc.tensor.matmul(pt[:, 0:hl],
                                     xti_sb[pp][:, :, t * 128:(t + 1) * 128],
                                     w_tiles[pp][:, :, ho:ho + hl],
                                     start=(pp == 0), stop=(pp == NP - 1),
                                     perf_mode=DR)
                nc.vector.tensor_tensor(
                    out_tiles[t][:, ho // 64:(ho + hl) // 64, :],
                    pt[:, 0:hl].rearrange("p (h k) -> p h k", k=64),
                    bvt[:, ho:ho + hl].rearrange("p (h k) -> p h k", k=64),
                    add_op)

            with tc.tile_pool(name="kqvg", bufs=1) as kqvg:
                kT_g = [kqvg.tile([128, S], bf16, tag=f"ktg{j}", name=f"ktg{j}")
                        for j in range(NHK)]
                qT_g = [kqvg.tile([128, SH], bf16, tag=f"qtg{j}", name=f"qtg{j}")
                        for j in range(NHK)]
                vx_g = [kqvg.tile([128, H, 64], bf16, tag=f"vxg{t}",
                                  name=f"vxg{t}") for t in range(NST)]

                with tc.tile_pool(name="ppa", bufs=2, space=PS) as ppa:
                    wk = load_w("wk_g")
                    for so, sl in _chunks(S, 512):
                        kq_chain(ppa, wk, 0, so, sl, kT_g[0], None)
                    wqg = load_w("wq_g")
                    for so, sl in _chunks(SH, 512):
                        kq_chain(ppa, wqg, 0, so, sl, qT_g[0],
                                 bcol_sb["bq_g"][0])

                    emit_scores(kT_g, qT_g, 0, g_rounds, 0)

                    for j in range(1, NHK):
                        for so, sl in _chunks(S, 512):
                            kq_chain(ppa, wk, j, so, sl, kT_g[j], None)
                        if j == 2:
                            emit_scores(kT_g, qT_g, 0, g_rounds, 1)
                    for j in range(1, NHK):
                        for so, sl in _chunks(SH, 512):
                            kq_chain(ppa, wqg, j, so, sl, qT_g[j],
                                     bcol_sb["bq_g"][j])
                    wv = load_w("wv_g")
                    for t in range(NST):
                        for ho, hl in _chunks(HK, 512):
                            v_chain(ppa, wv, t, ho, hl, vx_g, bv_bc["g"])
                    wvl = load_w("wv_l")
                    wkl = load_w("wk_l")

                o_g_sb = [ogp.tile([128, SH], bf16, tag=f"og{j}",
                                   name=f"og{j}") for j in range(NHK)]
                with tc.tile_pool(name="opg", bufs=1, space=PS) as opg, \
                     tc.tile_pool(name="rpg", bufs=1, space=PS) as rpg:
                    run_attention(kT_g, qT_g, vx_g, g_rounds, g_first, g_last,
                                  o_g_sb, 0, opg, rpg, skip_scores=WARMUP)

            with tc.tile_pool(name="kqvl", bufs=1) as kqvl:
                kT_l = [kqvl.tile([128, SH], bf16, tag=f"ktl{j}",
                                  name=f"ktl{j}") for j in range(NHK)]
                qT_l = [kqvl.tile([128, SH], bf16, tag=f"qtl{j}",
                                  name=f"qtl{j}") for j in range(NHK)]
                vx_l = [kqvl.tile([128, H, 64], bf16, tag=f"vxl{t}",
                                  name=f"vxl{t}") for t in range(NQT)]
                with tc.tile_pool(name="ppb", bufs=2, space=PS) as ppb:
                    for t in range(NQT):
                        for ho, hl in _chunks(HK, 512):
                            v_chain(ppb, wvl, t, ho, hl, vx_l, bv_bc["l"])
                    for j in range(NHK):
                        for so, sl in _chunks(SH, 512):
                            kq_chain(ppb, wkl, j, so, sl, kT_l[j], None)
                    wql = load_w("wq_l")
                    for j in range(NHK):
                        for so, sl in _chunks(SH, 512):
                            kq_chain(ppb, wql, j, so, sl, qT_l[j],
                                     bcol_sb["bq_l"][j])

                # wo prefetch during local phase (DMA engines idle here)
                wo_sb = []
                for t2 in range(NHK):
                    w = wop.tile([128, 2, D], fp8, tag=f"wo{t2}",
                                 name=f"wo{t2}")
                    nc.sync.dma_start(w[:], wo_d[t2])
                    wo_sb.append(w)

                o_l_sb = [olp.tile([128, SH], bf16, tag=f"ol{j}",
                                   name=f"ol{j}") for j in range(NHK)]
                with tc.tile_pool(name="opl", bufs=1, space=PS) as opl, \
                     tc.tile_pool(name="rpl", bufs=1, space=PS) as rpl:
                    run_attention(kT_l, qT_l, vx_l, l_rounds, l_first, l_last,
                                  o_l_sb, 4 * NHK, opl, rpl, lag=3)

        # ================= Phase D: normalize + out proj + LN ============
        with tc.tile_pool(name="oh", bufs=1) as ohp, \
             tc.tile_pool(name="yp", bufs=2, space=PS) as ypp, \
             tc.tile_pool(name="ln", bufs=2) as lnp:
            nslot = 6 * NHK
            rs_all = rip.tile([nslot, SH], f32, tag="rsall", name="rsall")
            ri_f = rip.tile([nslot, SH], f32, tag="rif", name="rif")
            ri_b = rip.tile([nslot, SH], bf16, tag="rib", name="rib")
            for r0, r1 in ((0, 2 * NHK), (4 * NHK, 6 * NHK)):
                nc.sync.dma_start(rs_all[r0:r1, :], rs_d[r0:r1, :])
                nc.vector.reciprocal_approx_fast(ri_f[r0:r1, :],
                                                 rs_all[r0:r1, :])
                # oh = (2/rs) * o_raw = 64 * o_norm -> fp8-friendly range
                nc.vector.tensor_scalar_mul(ri_b[r0:r1, :], ri_f[r0:r1, :],
                                            2.0)
                nc.sync.dma_start(ri_d[r0:r1, :], ri_b[r0:r1, :])

            oh_sb = []
            o_all = o_g_sb + o_l_sb
            for t2 in range(NHK):
                oh = ohp.tile([128, 2, SH], fp8, tag=f"oh{t2}", name=f"oh{t2}")
                for ko in range(2):
                    t = 2 * t2 + ko
                    slot = 2 * t if t < NHK else 4 * NHK + 2 * (t - NHK)
                    rb = ohp.tile([128, SH], bf16, tag="rb", name="rb", bufs=4)
                    for sub in range(2):
                        nc.sync.dma_start(
                            rb[sub * 64:sub * 64 + 64, :],
                            ri_d[slot + sub, :].rearrange("(a f) -> a f", a=1)
                            .partition_broadcast(64))
                    nc.vector.tensor_tensor(oh[:, ko, :], o_all[t][:], rb[:],
                                            mult_op)
                oh_sb.append(oh)

            gamma_bc = lnp.tile([128, D], f32, tag="gamma", name="gamma", bufs=1)
            nc.sync.dma_start(gamma_bc[:], gamma_d[:].partition_broadcast(128))
            beta_bc = lnp.tile([128, D], f32, tag="beta", name="beta", bufs=1)
            nc.sync.dma_start(beta_bc[:], beta_d[:].partition_broadcast(128))

            for qt in range(NQT):
                xq_t = lnp.tile([128, D], f32, tag="xq", name="xq")
                nc.sync.dma_start(xq_t[:], xq_d[qt * 128:(qt + 1) * 128, :])
                ps_y = ypp.tile([128, D], f32, tag="py", name="py")
                for do, dl in _chunks(D, 512):
                    for t2 in range(NHK):
                        nc.tensor.matmul(
                            ps_y[:, do:do + dl],
                            oh_sb[t2][:, :, qt * 128:(qt + 1) * 128],
                            wo_sb[t2][:, :, do:do + dl],
                            start=(t2 == 0), stop=(t2 == NHK - 1),
                            perf_mode=DR)
                y = lnp.tile([128, D], f32, tag="y", name="y")
                nc.vector.tensor_tensor(y[:], ps_y[:], xq_t[:], add_op)
                ssum = lnp.tile([128, 1], f32, tag="ssum", name="ssum")
                nc.vector.reduce_sum(ssum[:], y[:], axis=AxX)
                sqd = lnp.tile([128, D], bf16, tag="sqd", name="sqd")
                ssq = lnp.tile([128, 1], f32, tag="ssq", name="ssq")
                nc.scalar.activation(sqd[:], y[:], Square, accum_out=ssq[:])
                mu = lnp.tile([128, 1], f32, tag="mu", name="mu")
                nc.vector.tensor_scalar_mul(mu[:], ssum[:], 1.0 / D)
                var = lnp.tile([128, 1], f32, tag="var", name="var")
                nc.vector.tensor_scalar_mul(var[:], ssq[:], 1.0 / D)
                mu2 = lnp.tile([128, 1], f32, tag="mu2", name="mu2")
                nc.vector.tensor_tensor(mu2[:], mu[:], mu[:], mult_op)
                nc.vector.tensor_tensor(var[:], var[:], mu2[:], sub_op)
                sd = lnp.tile([128, 1], f32, tag="sd", name="sd")
                nc.scalar.activation(sd[:], var[:], Sqrt, bias=eps_col[:])
                rstd = lnp.tile([128, 1], f32, tag="rstd", name="rstd")
                nc.vector.reciprocal(rstd[:], sd[:])
                bco = lnp.tile([128, 1], f32, tag="bco", name="bco")
                nc.vector.tensor_tensor(bco[:], mu[:], rstd[:], mult_op)
                nc.vector.tensor_scalar_mul(bco[:], bco[:], -1.0)
                t1 = lnp.tile([128, D], f32, tag="t1", name="t1")
                nc.vector.tensor_scalar(t1[:], y[:], rstd[:], bco[:],
                                        mult_op, add_op)
                t2_ = lnp.tile([128, D], f32, tag="t2", name="t2")
                nc.vector.tensor_tensor(t2_[:], t1[:], gamma_bc[:], mult_op)
                ot = lnp.tile([128, D], f32, tag="ot", name="ot")
                nc.vector.tensor_tensor(ot[:], t2_[:], beta_bc[:], add_op)
                nc.sync.dma_start(out_d[qt * 128:(qt + 1) * 128, :], ot[:])

    nc.compile()
    return nc


def _pair_interleave(a):
    """[D, N] -> [D/256, 128, 2, N] with row (pair*256 + ko*128 + p)."""
    Dd, Nn = a.shape
    return np.ascontiguousarray(
        a.reshape(Dd // 256, 2, 128, Nn).transpose(0, 2, 1, 3))


def make_in_maps(inputs, cfg=None):
    cfg = dict(cfg or FULL_CFG)
    S, D, H, K = cfg["S"], cfg["D"], cfg["H"], cfg["K"]
    HK = H * K
    SH = S // 2
    NHK = HK // 128

    def np32(a):
        return np.asarray(a, dtype=np.float32)

    shared = {}
    for nm, key in (("wq_g", "gWq"), ("wk_g", "gWk"), ("wv_g", "gWv"),
                    ("wq_l", "lWq"), ("wk_l", "lWk"), ("wv_l", "lWv")):
        w = np32(inputs[key]).reshape(D, HK) * WSCALE
        shared[nm] = _pair_interleave(w).astype(FP8)
    wo = np.concatenate([np32(inputs["gWo"]).reshape(HK, D),
                         np32(inputs["lWo"]).reshape(HK, D)], axis=0) * WSCALE
    shared["wo"] = _pair_interleave(wo).astype(FP8)
    for nm, key in (("bq_g", "gbq"), ("bq_l", "lbq")):
        shared[nm] = np.ascontiguousarray(
            np32(inputs[key]).reshape(NHK, 128)) * WSCALE
    shared["bv_g"] = (np32(inputs["gbv"]).reshape(1, HK) * WSCALE).astype(BF16)
    shared["bv_l"] = (np32(inputs["lbv"]).reshape(1, HK) * WSCALE).astype(BF16)
    shared["gamma"] = np32(inputs["gamma"]).reshape(1, D)
    shared["beta"] = np32(inputs["beta"]).reshape(1, D)

    x = np32(inputs["x"])
    in_maps = []
    for c in range(N_CORES):
        b, half = divmod(c, 2)
        xb = x[b]
        xperm = np.concatenate([xb[half * SH:(half + 1) * SH],
                                xb[(1 - half) * SH:(2 - half) * SH]], axis=0)
        m = dict(shared)
        m["xti"] = _pair_interleave(
            np.ascontiguousarray(xperm.T)).astype(FP8)
        m["xq"] = np.ascontiguousarray(xperm[0:SH]) * YSCALE
        in_maps.append(m)
    return in_maps


def assemble_out(results, cfg=None):
    cfg = dict(cfg or FULL_CFG)
    S, D = cfg["S"], cfg["D"]
    SH = S // 2
    B = N_CORES // 2
    out = np.empty((B, S, D), np.float32)
    for c in range(N_CORES):
        b, half = divmod(c, 2)
        out[b, half * SH:(half + 1) * SH] = results[c]["out"]
    return out


_NC_CACHE = {}


def kernel(**inputs):
    from concourse.bass_utils import run_bass_kernel_spmd
    if "nc" not in _NC_CACHE:
        _NC_CACHE["nc"] = build_nc()
    nc = _NC_CACHE["nc"]
    in_maps = make_in_maps(inputs)
    res = run_bass_kernel_spmd(nc, in_maps, list(range(N_CORES)))
    return assemble_out(res.results)
